# revision 16
# baseline (speedup 1.0000x reference)
"""Noisy top-k (k=2) router for Trainium2, data-parallel over 8 NeuronCores.

Math: for each row of x = logits + noise with top-2 values v1 >= v2, the
top-2 softmax weights are sigmoid(+-(v1-v2)), and for any element
    out = (x >= v2) * sigmoid(2x - (v1+v2))
equals the scattered result exactly (selection compares run on exact fp32
values, so this matches the reference whenever no row has a v2 == v3 tie).

v5 structure (from measured traces):
  - DMA chunks of 32 rows/partition (8KB descriptors - smaller packets
    halve per-packet DMA efficiency): lg loads on the SP HWDGE ring,
    nz SWDGE accum-adds (CCE) on GpSimd, stores on the ACT ring.
  - compute chunks of 16 rows = half a DMA chunk.
  - DVE runs the MAX8 stream nearly uninterrupted. The per-chunk s=v1+v2
    and mask=(x>=v2) DVE ops are emitted ONE CHUNK LATE: a same-engine
    sem-count wait otherwise stalls DVE ~1.1us per chunk (semaphore
    updates lag execution; by the next burst's end they have propagated).
  - sigmoid hybrid: route A: u = x - (v1+v2)/2 on GpSimd + one big ACT
    sigmoid(scale=2); route B: per-row ACT sigmoid bias=-(v1+v2).
  - combine out = sig*mask on GpSimd (2 tail chunks on DVE), two chunks
    late; stores once both halves of a DMA chunk are combined.
This walrus codegen allows only ONE sync-wait per instruction; the
_legalize_waits post-pass splits any excess into standalone EventSemaphore
instructions (which hold two).
"""

import time

import numpy as np

import concourse.bass as bass
import concourse.mybir as mybir
from concourse.tile import TileContext
from concourse.bass_utils import run_bass_kernel_spmd

B = 262144
E = 64
N_CORES = 8
B_CORE = B // N_CORES  # 32768 rows per core

P = 128  # SBUF partitions
ROWS_PP = B_CORE // P  # 256 rows per partition

RD = 32  # rows per partition per DMA chunk
ND = ROWS_PP // RD  # 8 DMA chunks
RC = 16  # rows per partition per compute chunk
NCH = ROWS_PP // RC  # 16 compute chunks (2 per DMA chunk)

# per-compute-chunk sigmoid route: 'A' = u-pass (GpSimd) + one big ACT
# sigmoid; 'B' = per-row biased ACT sigmoid (no GpSimd work)
ROUTES = "BBBBABABABABBBBB"
assert len(ROUTES) == NCH and ROUTES.count("B") == 12

# compute chunks whose combine runs on DVE instead of GpSimd
COMB_DVE = ()
# DMA chunks whose add runs as a DVE TT instead of SWDGE accum (ramp)
ADD_DVE = (0, 1, 2)

_CACHE = {}

# test.py introspection: BassKernelResults of the most recent run
LAST_RESULT = None


def _legalize_waits(nc: "bass.Bass") -> None:
    """This walrus codegen accepts at most ONE sync-wait per instruction
    (two on EventSemaphore). Tile's wait assigner can emit more; split the
    excess into standalone EventSemaphore waits placed immediately before
    the instruction on the same engine (identical semantics: the engine
    blocks there instead)."""
    n = 0
    for fnb in nc.m.functions[0].blocks:
        out = []
        for inst in fnb.instructions:
            si = inst.sync_info
            cap = 2 if isinstance(inst, mybir.InstEventSemaphore) else 1
            if si is not None and len(si.on_wait) > cap:
                waits = list(si.on_wait)
                extra, keep = waits[:-cap], waits[-cap:]
                for c in range(0, len(extra), 2):
                    n += 1
                    out.append(
                        mybir.InstEventSemaphore(
                            name=f"EVW-{n}",
                            engine=inst.engine,
                            sync_info=mybir.SyncInfo(
                                on_wait=extra[c : c + 2], on_update=[]
                            ),
                        )
                    )
                inst.sync_info = mybir.SyncInfo(
                    on_wait=keep, on_update=list(si.on_update)
                )
            out.append(inst)
        fnb.instructions = out


def _build_nc() -> bass.Bass:
    nc = bass.Bass()
    f32 = mybir.dt.float32
    TT = mybir.AluOpType
    AF = mybir.ActivationFunctionType

    lg = nc.dram_tensor("logits", [B_CORE, E], f32, kind="ExternalInput")
    nz = nc.dram_tensor("noise", [B_CORE, E], f32, kind="ExternalInput")
    out = nc.dram_tensor("out", [B_CORE, E], f32, kind="ExternalOutput")

    # partition-major: partition p owns ROWS_PP contiguous DRAM rows; one
    # DMA chunk is an 8KB-contiguous descriptor per partition
    lgv = lg[:].rearrange("(p d r) e -> d p r e", p=P, d=ND)
    nzv = nz[:].rearrange("(p d r) e -> d p r e", p=P, d=ND)
    outv = out[:].rearrange("(p d r) e -> d p r e", p=P, d=ND)

    with TileContext(nc) as tc:
        with (
            tc.tile_pool(name="big", bufs=4) as big_pool,
            tc.tile_pool(name="sm", bufs=6) as sm_pool,
        ):

            def issue_dma(d):
                """load lg DMA chunk d and add nz into it"""
                xd = big_pool.tile([P, RD, E], f32, tag="x", bufs=6)
                if d in ADD_DVE:
                    lgt = big_pool.tile([P, RD, E], f32, tag="lgt", bufs=2)
                    nzt = big_pool.tile([P, RD, E], f32, tag="nzt", bufs=2)
                    nc.sync.dma_start(out=lgt, in_=lgv[d])
                    nc.sync.dma_start(out=nzt, in_=nzv[d])
                    nc.vector.tensor_tensor(
                        out=xd, in0=lgt, in1=nzt, op=TT.add
                    )
                else:
                    nc.sync.dma_start(out=xd, in_=lgv[d])
                    nc.gpsimd.dma_start(
                        out=xd, in_=nzv[d], accum_op=TT.add
                    )
                return xd

            xd = [None] * ND  # DMA-chunk x tiles
            otd = [None] * ND  # DMA-chunk output tiles
            v8s = [None] * NCH
            pend = [None] * NCH  # (x, sg, m, ot_slice) awaiting combine

            def x_of(c):
                return xd[c // 2][:, (c % 2) * RC : (c % 2) * RC + RC]

            def emit_max8(c):
                x = x_of(c)
                v8 = sm_pool.tile([P, RC, 8], f32, tag="v8", bufs=8)
                for r in range(RC):
                    nc.vector.max(out=v8[:, r, :], in_=x[:, r, :])
                v8s[c] = v8

            def emit_tail(c):
                """s, mask, sigmoid for chunk c (one chunk late so the DVE
                sem-count waits are pre-satisfied)"""
                x = x_of(c)
                v8 = v8s[c]
                s = sm_pool.tile([P, RC], f32, tag="s")
                nc.vector.tensor_tensor(
                    out=s, in0=v8[:, :, 0], in1=v8[:, :, 1], op=TT.add
                )
                m = big_pool.tile([P, RC, E], f32, tag="m", bufs=5)
                nc.vector.tensor_tensor(
                    out=m,
                    in0=x,
                    in1=v8[:, :, 1].to_broadcast([P, RC, E]),
                    op=TT.is_ge,
                )
                sg = big_pool.tile([P, RC, E], f32, tag="sg", bufs=5)
                if ROUTES[c] == "A":
                    sh = sm_pool.tile([P, RC], f32, tag="sh")
                    nc.scalar.activation(
                        out=sh, in_=s, func=AF.Copy, scale=0.5
                    )
                    u = big_pool.tile([P, RC, E], f32, tag="u")
                    nc.gpsimd.tensor_tensor(
                        out=u,
                        in0=x,
                        in1=sh.to_broadcast([P, RC, E]),
                        op=TT.subtract,
                    )
                    nc.scalar.activation(
                        out=sg, in_=u, func=AF.Sigmoid, scale=2.0
                    )
                else:
                    negs = sm_pool.tile([P, RC], f32, tag="ng")
                    nc.scalar.activation(
                        out=negs, in_=s, func=AF.Copy, scale=-1.0
                    )
                    for r in range(RC):
                        nc.scalar.activation(
                            out=sg[:, r, :],
                            in_=x[:, r, :],
                            func=AF.Sigmoid,
                            bias=negs[:, r : r + 1],
                            scale=2.0,
                        )
                d = c // 2
                if c % 2 == 0:
                    otd[d] = big_pool.tile(
                        [P, RD, E], f32, tag="ot", bufs=3, name=f"ot{d}"
                    )
                ot = otd[d][:, (c % 2) * RC : (c % 2) * RC + RC]
                pend[c] = (sg, m, ot)

            def emit_comb(c):
                sg, m, ot = pend[c]
                eng = nc.vector if c in COMB_DVE else nc.gpsimd
                eng.tensor_tensor(out=ot, in0=sg, in1=m, op=TT.mult)

            def emit_store(d):
                nc.scalar.dma_start(out=outv[d], in_=otd[d])

            xd[0] = issue_dma(0)
            xd[1] = issue_dma(1)
            xd[2] = issue_dma(2)
            for c in range(NCH):
                if c % 2 == 0 and c // 2 + 3 < ND:
                    xd[c // 2 + 3] = issue_dma(c // 2 + 3)
                emit_max8(c)
                if c >= 1:
                    emit_tail(c - 1)
                if c >= 2:
                    emit_comb(c - 2)
                if c >= 4 and c % 2 == 0:
                    emit_store((c - 4) // 2)
            # drain
            emit_tail(NCH - 1)
            emit_store(ND - 2)
            emit_comb(NCH - 2)
            emit_comb(NCH - 1)
            emit_store(ND - 1)

    _legalize_waits(nc)
    return nc


def _get_nc() -> bass.Bass:
    if "nc" not in _CACHE:
        _CACHE["nc"] = _build_nc()
    return _CACHE["nc"]


def kernel(logits: np.ndarray, noise: np.ndarray) -> np.ndarray:
    global LAST_RESULT
    logits = np.ascontiguousarray(np.asarray(logits), dtype=np.float32)
    noise = np.ascontiguousarray(np.asarray(noise), dtype=np.float32)
    assert logits.shape == (B, E) and noise.shape == (B, E)

    lg_shards = np.split(logits, N_CORES, axis=0)
    nz_shards = np.split(noise, N_CORES, axis=0)
    in_maps = [
        {"logits": lg_shards[i], "noise": nz_shards[i]} for i in range(N_CORES)
    ]

    try:
        res = run_bass_kernel_spmd(
            _get_nc(), in_maps, core_ids=list(range(N_CORES))
        )
    except Exception:
        # transient NRT device errors have been observed right after a
        # crashed run; one retry clears them
        time.sleep(5)
        res = run_bass_kernel_spmd(
            _get_nc(), in_maps, core_ids=list(range(N_CORES))
        )
    LAST_RESULT = res
    return np.concatenate([r["out"] for r in res.results], axis=0)
